# revision 31
# baseline (speedup 1.0000x reference)
import sys

for _p in ("/opt/trn_rl_repo", "/root/.axon_site/_ro/trn_rl_repo"):
    if _p not in sys.path:
        sys.path.insert(0, _p)

import numpy as np
import ml_dtypes

from concourse import bass, bacc, mybir
from concourse.tile import TileContext
from concourse.bass_utils import run_bass_kernel_spmd

BF16 = ml_dtypes.bfloat16

# ---- problem constants (hardcoded per contract) ----
B, T, NB, D = 8, 2048, 22, 128
WIDTH = 64
FREQ = 1025
N_FFT = 2048
HOP = 512
STARTS = [0, 48, 96, 144, 192, 240, 288, 336, 384, 432, 480, 528,
          576, 624, 672, 720, 768, 816, 864, 912, 960, 961]
NCHUNK = 8          # full 128-bin frequency chunks (bins 0..1023); bin 1024 separate
TT = 512            # time-tile width
NTT = T // TT       # 4 time tiles
OUTC = 2047         # output columns: out[512*c + r], c in [0, 2047), r in [0,512)
N_CORES = 8

# band->chunk incidence: for chunk k, list of band indices overlapping bins [128k, 128k+128)
def _incidence():
    inc = []
    for k in range(NCHUNK):
        lo_bin, hi_bin = 128 * k, 128 * k + 128
        bands = []
        for n, s in enumerate(STARTS):
            lo, hi = max(lo_bin, s), min(hi_bin, s + WIDTH)
            if lo < hi and not (n == 21 and lo_bin <= 1024 < hi_bin):
                bands.append(n)
        inc.append(bands)
    return inc

INC = _incidence()

# packed weight-block column offsets: blocks ordered (k, comp, band)
def _wblock_layout():
    off = 0
    layout = {}  # (k, comp, n) -> col offset (128 cols each)
    for k in range(NCHUNK):
        for comp in range(2):
            for n in INC[k]:
                layout[(k, comp, n)] = off
                off += 128
    # bin-1024 blocks: band 21, w=63; 1 col each for comp 0/1
    layout[(8, 0, 21)] = off
    layout[(8, 1, 21)] = off + 1
    off += 2
    return layout, off

WLAYOUT, WCOLS = _wblock_layout()

_CACHE = {}


def _build_nc():
    f32 = mybir.dt.float32
    bf16 = mybir.dt.bfloat16
    nc = bacc.Bacc(None, target_bir_lowering=False, debug=False)

    zp = nc.dram_tensor("zp", [NTT, 128, NB, TT], bf16, kind="ExternalInput")
    mixp = nc.dram_tensor("mixp", [NTT, NCHUNK, 128, 2, TT], bf16, kind="ExternalInput")
    mix8 = nc.dram_tensor("mix8", [1, 2, T], bf16, kind="ExternalInput")
    mp = nc.dram_tensor("mp", [128, NCHUNK, 2, FREQ], bf16, kind="ExternalInput")
    mp8 = nc.dram_tensor("mp8", [1, FREQ], bf16, kind="ExternalInput")
    wb = nc.dram_tensor("wb", [128, WCOLS], bf16, kind="ExternalInput")
    biasv = nc.dram_tensor("biasv", [128, NCHUNK + 1, 2], f32, kind="ExternalInput")
    edge = nc.dram_tensor("edge", [128, 4, 2], f32, kind="ExternalInput")
    winv_d = nc.dram_tensor("winv", [128, 16], f32, kind="ExternalInput")
    jrev_d = nc.dram_tensor("jrev", [128, 128], bf16, kind="ExternalInput")
    e00_d = nc.dram_tensor("e00", [1, 128], bf16, kind="ExternalInput")
    outp = nc.dram_tensor("outp", [4, 128, OUTC], f32, kind="ExternalOutput")

    with TileContext(nc) as tc:
        with (
            tc.tile_pool(name="singles", bufs=1) as singles,
            tc.tile_pool(name="zpool", bufs=2) as zpool,
            tc.tile_pool(name="mixpool", bufs=4) as mixpool,
            tc.tile_pool(name="mix8pool", bufs=2) as mix8pool,
            tc.tile_pool(name="spec", bufs=19) as specpool,
            tc.tile_pool(name="mrmi", bufs=4) as mrmipool,
            tc.tile_pool(name="tmp", bufs=6) as tmppool,
            tc.tile_pool(name="ev", bufs=5) as evpool,
            tc.tile_pool(name="fmp", bufs=4) as fmpool,
            tc.tile_pool(name="fpp", bufs=5) as fppool,
            tc.tile_pool(name="maskps", bufs=2, space="PSUM") as maskpool,
            tc.tile_pool(name="dftps", bufs=4, space="PSUM") as dftpool,
            tc.tile_pool(name="revps", bufs=2, space="PSUM") as revpool,
        ):
            # DMA order = need order: wb/z/mix feed the first matmuls, the
            # basis halves arrive just before the first P/Q chains need them
            wb_t = singles.tile([128, WCOLS], bf16, tag="wb")
            nc.sync.dma_start(wb_t[:], wb[:])
            z0_t = zpool.tile([128, NB, TT], bf16, tag="z")
            nc.scalar.dma_start(z0_t[:].rearrange("p a b -> p (a b)"), zp[0].rearrange("p a b -> p (a b)"))
            biasv_t = singles.tile([128, NCHUNK + 1, 2], f32, tag="biasv")
            nc.sync.dma_start(biasv_t[:], biasv[:])
            mix_pref = {}
            for k in (0, 1, 2):
                mt = mixpool.tile([128, 2, TT], bf16, tag="mix")
                nc.gpsimd.dma_start(mt[:].rearrange("p a b -> p (a b)"), mixp[0, k].rearrange("p a b -> p (a b)"))
                mix_pref[k] = mt
            winv_t = singles.tile([128, 16], f32, tag="winv")
            nc.sync.dma_start(winv_t[:], winv_d[:])
            mp_t = singles.tile([128, NCHUNK, 2, FREQ], bf16, tag="mp")
            nc.sync.dma_start(mp_t[:, :, 0, :], mp[:, :, 0, :])
            mp8_t = singles.tile([1, FREQ], bf16, tag="mp8")
            nc.sync.dma_start(mp8_t[:], mp8[:])
            nc.sync.dma_start(mp_t[:, :, 1, :], mp[:, :, 1, :])
            edge_t = singles.tile([128, 4, 2], f32, tag="edge")
            nc.sync.dma_start(edge_t[:], edge[:])
            jrev_t = singles.tile([128, 128], bf16, tag="jrev")
            nc.sync.dma_start(jrev_t[:], jrev_d[:])
            e00_t = singles.tile([1, 128], bf16, tag="e00")
            nc.sync.dma_start(e00_t[:], e00_d[:])

            outs = []
            for u in range(4):
                o = singles.tile([128, OUTC], f32, tag=f"out{u}")
                nc.vector.memset(o[:], 0.0)
                outs.append(o)

            ident = mybir.ActivationFunctionType.Identity

            def ola_range(t0, q):
                a = t0 + q - 2
                fa = 0
                if a < 0:
                    fa = -a
                    a = 0
                b_ = t0 + q - 2 + TT
                fb = TT
                if b_ > OUTC:
                    fb = TT - (b_ - OUTC)
                    b_ = OUTC
                return a, b_, fa, fb

            for tau in range(NTT):
                t0 = tau * TT
                if tau == 0:
                    ztile = z0_t
                else:
                    ztile = zpool.tile([128, NB, TT], bf16, tag="z")
                    nc.scalar.dma_start(ztile[:].rearrange("p a b -> p (a b)"), zp[tau].rearrange("p a b -> p (a b)"))

                spec_r, spec_i = [], []
                for k in range(NCHUNK + 1):
                    npart = 128 if k < NCHUNK else 1
                    bands = INC[k] if k < NCHUNK else [21]
                    if k < NCHUNK:
                        if tau == 0 and k in mix_pref:
                            mtile = mix_pref[k]
                        else:
                            mtile = mixpool.tile([128, 2, TT], bf16, tag="mix")
                            nc.gpsimd.dma_start(mtile[:].rearrange("p a b -> p (a b)"), mixp[tau, k].rearrange("p a b -> p (a b)"))
                    else:
                        mtile = mix8pool.tile([1, 2, TT], bf16, tag="mix8")
                        nc.sync.dma_start(mtile[:], mix8[:, :, t0:t0 + TT])
                    mxr = mtile[:npart, 0, :]
                    mxi = mtile[:npart, 1, :]
                    ps_pair = []
                    for comp in range(2):
                        ps = maskpool.tile([npart, TT], f32, tag="maskps")
                        for bi, n in enumerate(bands):
                            coloff = WLAYOUT[(k, comp, n)]
                            nc.tensor.matmul(
                                ps[:npart, :],
                                wb_t[:, coloff:coloff + npart],
                                ztile[:, n, :],
                                start=(bi == 0),
                                stop=(bi == len(bands) - 1),
                            )
                        ps_pair.append(ps)
                    ps_r, ps_i = ps_pair
                    # evacuate PSUM with the bias fold, cast to bf16
                    mr = mrmipool.tile([npart, TT], bf16, tag="mrmi")
                    nc.scalar.activation(mr[:npart, :], ps_r[:npart, :], ident,
                                         bias=biasv_t[:npart, k, 0:1],
                                         scale=1.0)
                    mi = mrmipool.tile([npart, TT], bf16, tag="mrmi")
                    nc.scalar.activation(mi[:npart, :], ps_i[:npart, :], ident,
                                         bias=biasv_t[:npart, k, 1:2],
                                         scale=1.0)
                    # spec = mask * mix (complex)
                    p1 = tmppool.tile([npart, TT], bf16, tag="tmp")
                    nc.vector.tensor_mul(p1[:npart, :], mr[:npart, :], mxr)
                    p2 = tmppool.tile([npart, TT], bf16, tag="tmp")
                    nc.vector.tensor_mul(p2[:npart, :], mi[:npart, :], mxi)
                    sr = specpool.tile([npart, TT], bf16, tag="spec")
                    nc.vector.tensor_sub(sr[:npart, :], p1[:npart, :], p2[:npart, :])
                    spec_r.append(sr)
                    if k < NCHUNK:
                        p3 = tmppool.tile([npart, TT], bf16, tag="tmp")
                        nc.vector.tensor_mul(p3[:npart, :], mr[:npart, :], mxi)
                        p4 = tmppool.tile([npart, TT], bf16, tag="tmp")
                        nc.vector.tensor_mul(p4[:npart, :], mi[:npart, :], mxr)
                        si = specpool.tile([npart, TT], bf16, tag="spec")
                        nc.vector.tensor_add(si[:npart, :], p3[:npart, :], p4[:npart, :])
                        spec_i.append(si)
                    else:
                        spec_i.append(None)

                # s = 1024 singleton: frames[1024] = P[1024] (Q[1024]=0, win=1)
                ps1024 = dftpool.tile([1, TT], f32, tag="dftps")
                for k in range(NCHUNK + 1):
                    kp = 128 if k < NCHUNK else 1
                    lhs = (mp_t[:, k, 0, 1024:1025] if k < NCHUNK
                           else mp8_t[:1, 1024:1025])
                    nc.tensor.matmul(ps1024[:1, :], lhs,
                                     spec_r[k][:kp, :],
                                     start=(k == 0), stop=(k == NCHUNK))
                f1024 = fppool.tile([1, TT], bf16, tag="fpp")
                nc.scalar.copy(f1024[:1, :], ps1024[:1, :])

                # frequency-to-frame blocks, descending so the mirror OLA can
                # consume fp[blk] and fp[blk+1] row 0 as soon as they exist
                fps = [None] * 8
                for blk in range(7, -1, -1):
                    soff = 128 * blk
                    Pps = dftpool.tile([128, TT], f32, tag="dftps")
                    for k in range(NCHUNK + 1):
                        kp = 128 if k < NCHUNK else 1
                        lhs = (mp_t[:, k, 0, soff:soff + 128] if k < NCHUNK
                               else mp8_t[:1, soff:soff + 128])
                        nc.tensor.matmul(Pps[:], lhs,
                                         spec_r[k][:kp, :],
                                         start=(k == 0), stop=(k == NCHUNK))
                    Qps = dftpool.tile([128, TT], f32, tag="dftps")
                    for k in range(NCHUNK):
                        nc.tensor.matmul(Qps[:], mp_t[:, k, 1, soff:soff + 128],
                                         spec_i[k][:],
                                         start=(k == 0), stop=(k == NCHUNK - 1))
                    # evacuate with the (symmetric) Hann window folded in:
                    # win[2048-s] == win[s], so windowed P/Q serve both halves
                    qs = evpool.tile([128, TT], bf16, tag="ev")
                    nc.scalar.activation(qs[:], Qps[:], ident,
                                         scale=winv_t[:, blk:blk + 1])
                    pw = evpool.tile([128, TT], bf16, tag="ev")
                    nc.scalar.activation(pw[:], Pps[:], ident,
                                         scale=winv_t[:, blk:blk + 1])
                    # direct half: OUT += (P - Q)*win  (slack path -> gpsimd)
                    fm = fmpool.tile([128, TT], bf16, tag="fmp")
                    nc.gpsimd.tensor_sub(fm[:], pw[:], qs[:])
                    q_, u_ = blk // 4, blk % 4
                    a, b_, fa, fb = ola_range(t0, q_)
                    if fb > fa:
                        o = outs[u_]
                        nc.gpsimd.tensor_add(o[:, a:b_], o[:, a:b_], fm[:, fa:fb])
                    # mirrored half source: (P + Q)*win, reversal via PE
                    fp = fppool.tile([128, TT], bf16, tag="fpp")
                    nc.vector.tensor_add(fp[:], pw[:], qs[:])
                    fps[blk] = fp

                    # mirror block j = 15 - blk needs fp[blk] and fp[blk+1][0]
                    j = 15 - blk
                    Frev = revpool.tile([128, TT], f32, tag="revps")
                    nc.tensor.matmul(Frev[:], jrev_t[:], fp[:],
                                     start=True, stop=False)
                    bsrc = f1024 if j == 8 else fps[16 - j]
                    nc.tensor.matmul(Frev[:], e00_t[:1, :], bsrc[:1, :],
                                     start=False, stop=True)
                    q_, u_ = j // 4, j % 4
                    a, b_, fa, fb = ola_range(t0, q_)
                    if fb > fa:
                        o = outs[u_]
                        nc.vector.tensor_add(o[:, a:b_], o[:, a:b_], Frev[:, fa:fb])

                # columns [512*tau - 2, 512*tau + 510) receive no further
                # writes after this tile -- stream them out now
                lo = max(0, t0 - 2)
                hi = OUTC if tau == NTT - 1 else t0 + 510
                if tau == 0:
                    for u in range(4):
                        nc.vector.tensor_mul(outs[u][:, 0:1], outs[u][:, 0:1],
                                             edge_t[:, u, 0:1])
                if tau == NTT - 1:
                    for u in range(4):
                        nc.vector.tensor_mul(
                            outs[u][:, OUTC - 1:OUTC],
                            outs[u][:, OUTC - 1:OUTC], edge_t[:, u, 1:2])
                for u in range(4):
                    nc.scalar.dma_start(outp[u][:, lo:hi], outs[u][:, lo:hi])

    if not nc.is_finalized():
        nc.finalize()
    return nc


def _host_constants():
    # overlap counts per frequency bin
    wgt = np.zeros(FREQ, np.float64)
    for n, s in enumerate(STARTS):
        wgt[s:s + WIDTH] += 1.0
    wgt = np.maximum(wgt, 1.0)

    # scaled IDFT basis, s in [0, 1025), window NOT folded (applied via STT scale)
    # frames[s] = P[s] - Q[s];  frames[2048-s] = P[s] + Q[s]
    s_idx = np.arange(N_FFT)
    win = 0.5 * (1.0 - np.cos(2.0 * np.pi * s_idx / N_FFT))
    f_idx = np.arange(FREQ)
    c_f = np.full(FREQ, 2.0)
    c_f[0] = 1.0
    c_f[N_FFT // 2] = 1.0
    sh = np.arange(FREQ)  # s in [0, 1025)
    ang = 2.0 * np.pi * np.outer(f_idx, sh) / N_FFT
    scale = (c_f / (N_FFT * 1.5))[:, None]
    Mc = np.cos(ang) * scale
    Ms = np.sin(ang) * scale
    mp = np.zeros((128, NCHUNK, 2, FREQ), np.float64)
    for k in range(NCHUNK):
        mp[:, k, 0, :] = Mc[128 * k:128 * k + 128]
        mp[:, k, 1, :] = Ms[128 * k:128 * k + 128]
    mp_bf = mp.astype(BF16)
    mp8_bf = Mc[1024][None, :].astype(BF16)
    # window scale vectors (f32): winv[p, b] = win[128b+p], b in [0,16)
    # (mirror blocks use the same direct window since scaling happens after
    # partition reversal: block j covers output s' = 128j+p)
    winv = np.zeros((128, 16), np.float32)
    for bb in range(16):
        winv[:, bb] = win[128 * bb + np.arange(128)]
    # partition-reversal permutation: out[p'] = in[128-p'] for p' in [1,128)
    jrev = np.zeros((128, 128), np.float64)
    for p in range(1, 128):
        jrev[p, 128 - p] = 1.0
    jrev_bf = jrev.astype(BF16)
    e00 = np.zeros((1, 128), np.float64)
    e00[0, 0] = 1.0
    e00_bf = e00.astype(BF16)

    # edge ratios for the two output columns with only 3 overlapping frames
    w2 = win * win
    env0 = w2[np.arange(512)] + w2[512 + np.arange(512)] + w2[1024 + np.arange(512)]
    envL = w2[512 + np.arange(512)] + w2[1024 + np.arange(512)] + w2[1536 + np.arange(512)]
    edge = np.zeros((128, 4, 2), np.float32)
    for u in range(4):
        r = 128 * u + np.arange(128)
        edge[:, u, 0] = (1.5 / env0[r]).astype(np.float32)
        edge[:, u, 1] = (1.5 / envL[r]).astype(np.float32)
    return wgt, mp_bf, mp8_bf, edge, winv, jrev_bf, e00_bf


def _pack_weights(W, b, wgt):
    # de-interleave + fold 1/wgt:  W2[n,d,w] (real), W2[n,d,64+w] (imag)
    W = np.asarray(W, np.float64)
    b = np.asarray(b, np.float64)
    W2 = np.zeros((NB, D, 128), np.float64)
    for n, s in enumerate(STARTS):
        g = wgt[s:s + WIDTH]
        W2[n, :, :WIDTH] = W[n, :, 0::2] / g[None, :]
        W2[n, :, WIDTH:] = W[n, :, 1::2] / g[None, :]
    wbp = np.zeros((128, WCOLS), np.float64)
    for key, off in WLAYOUT.items():
        k, comp, n = key
        s = STARTS[n]
        if k < NCHUNK:
            blk = np.zeros((D, 128), np.float64)
            for j in range(128):
                w = 128 * k + j - s
                if 0 <= w < WIDTH:
                    blk[:, j] = W2[n, :, comp * WIDTH + w]
            wbp[:, off:off + 128] = blk
        else:
            wbp[:, off] = W2[n, :, comp * WIDTH + 63]
    # bias vector per (k, comp): summed overlapping-band biases / wgt
    bias_f = np.zeros((FREQ, 2), np.float64)
    for f in range(FREQ):
        for n, s in enumerate(STARTS):
            w = f - s
            if 0 <= w < WIDTH:
                bias_f[f, 0] += b[n, 2 * w]
                bias_f[f, 1] += b[n, 2 * w + 1]
        bias_f[f] /= wgt[f]
    biasv = np.zeros((128, NCHUNK + 1, 2), np.float32)
    for k in range(NCHUNK):
        biasv[:, k, 0] = bias_f[128 * k:128 * k + 128, 0]
        biasv[:, k, 1] = bias_f[128 * k:128 * k + 128, 1]
    biasv[0, 8, 0] = bias_f[1024, 0]
    biasv[0, 8, 1] = bias_f[1024, 1]
    return wbp.astype(BF16), biasv


def kernel(z, mix_spec, W, b):
    if "nc" not in _CACHE:
        _CACHE["nc"] = _build_nc()
        _CACHE["consts"] = _host_constants()
    nc = _CACHE["nc"]
    wgt, mp_bf, mp8_bf, edge, winv, jrev_bf, e00_bf = _CACHE["consts"]
    wbp, biasv = _pack_weights(W, b, wgt)

    in_maps = []
    for core in range(N_CORES):
        # z: (T, NB, D) -> (NTT, D=128, NB, TT)
        zb = np.transpose(z[core], (2, 1, 0)).reshape(128, NB, NTT, TT)
        zb = np.ascontiguousarray(np.transpose(zb, (2, 0, 1, 3))).astype(BF16)
        mx = mix_spec[core]  # (2, T, FREQ)
        mxT = np.transpose(mx, (0, 2, 1))  # (2, FREQ, T)
        mixp = np.zeros((NTT, NCHUNK, 128, 2, TT), BF16)
        for k in range(NCHUNK):
            blkv = mxT[:, 128 * k:128 * k + 128, :].reshape(2, 128, NTT, TT)
            mixp[:, k] = np.transpose(blkv, (2, 1, 0, 3)).astype(BF16)
        mix8v = np.zeros((1, 2, T), BF16)
        mix8v[0, 0, :] = mxT[0, 1024].astype(BF16)
        mix8v[0, 1, :] = mxT[1, 1024].astype(BF16)
        in_maps.append({
            "zp": zb,
            "mixp": mixp,
            "mix8": mix8v,
            "mp": mp_bf,
            "mp8": mp8_bf,
            "wb": wbp,
            "biasv": biasv,
            "edge": edge,
            "winv": winv,
            "jrev": jrev_bf,
            "e00": e00_bf,
        })

    res = run_bass_kernel_spmd(nc, in_maps, core_ids=list(range(N_CORES)))
    out = np.empty((B, HOP * (T - 1)), np.float32)
    for core in range(N_CORES):
        o = res.results[core]["outp"]  # (4, 128, OUTC)
        out[core] = np.ascontiguousarray(np.transpose(o, (2, 0, 1))).reshape(-1)
    return out
